# revision 5
# baseline (speedup 1.0000x reference)
"""Trainium2 Bass kernel for nn_DGG_StraightThrough.

The reference's pairwise-logit MLP is mathematically dead: softmax over the
singleton feature dim is identically 1, so log_p == 0 and the gumbel logits
y equal `temp` exactly (bit-for-bit, verified).  The output is therefore the
straight-through top-8 row indicator of temp, identical for every batch
entry:  adj[b,i,j] = 1.0 iff temp[i,j] is among the 8 largest of row i.

Sharding: row-parallel over N=2048 across 8 cores (256 rows each).  Each
core: DMA its [256,2048] slab in, DVE Max8 for the per-row 8th-largest
threshold, is_ge compare to build the 0/1 mask, DMA the mask out.  Host
concatenates the 8 slabs and broadcasts over B=4.
"""

import sys

import numpy as np

if "/opt/trn_rl_repo" not in sys.path:
    sys.path.insert(0, "/opt/trn_rl_repo")

B, N, K = 4, 2048, 8
N_CORES = 8
ROWS = N // N_CORES  # 256 rows per core
P = 128  # SBUF partitions

# Hooks for a driving harness (test.py): extra kwargs for run_bass_kernel_spmd
# and the last BassKernelResults (exec_time_ns etc).
RUN_KWARGS: dict = {}
LAST_RESULT = None

_PROGRAM = None


def _build_program():
    import concourse.bass as bass
    import concourse.mybir as mybir

    nc = bass.Bass()
    t_in = nc.declare_dram_parameter("t", [ROWS, N], mybir.dt.float32, isOutput=False)
    out = nc.declare_dram_parameter("out", [ROWS, N], mybir.dt.float32, isOutput=True)

    nblk = ROWS // P  # row-blocks of 128 stacked along the free dim
    # partition p holds rows {p, P+p, ...}: [ROWS, N] -> [P, nblk, N]
    t_view = t_in.rearrange("(b p) n -> p b n", p=P)
    o_view = out.rearrange("(b p) n -> p b n", p=P)

    with (
        nc.sbuf_tensor([P, nblk * N], mybir.dt.float32) as tile,
        nc.sbuf_tensor([P, nblk * N], mybir.dt.float32) as mask,
        nc.sbuf_tensor([P, 8 * nblk], mybir.dt.float32) as top8,
        nc.semaphore("dma_sem") as dma_sem,
        nc.semaphore("v_sem") as v_sem,
        nc.semaphore("mx_sem") as mx_sem,
        nc.Block() as block,
    ):

        @block.sync
        def _(sync):
            # single input DMA: one InstDMACopy fans out over all 16 SDMA engines
            sync.dma_start(
                out=tile[:].rearrange("p (b n) -> p b n", b=nblk), in_=t_view
            ).then_inc(dma_sem, 16)
            sync.wait_ge(v_sem, nblk)
            sync.dma_start(
                out=o_view, in_=mask[:].rearrange("p (b n) -> p b n", b=nblk)
            ).then_inc(dma_sem, 16)
            sync.wait_ge(dma_sem, 32)

        @block.vector
        def _(vector):
            vector.wait_ge(dma_sem, 16)
            for b in range(nblk):
                # then_inc/wait: the scalar-ptr operand of tensor_scalar is
                # fetched early, racing the in-pipeline Max write on the same
                # engine — needs an explicit sem hop (CoreSim race detector).
                vector.max(
                    top8[:, 8 * b : 8 * (b + 1)], tile[:, b * N : (b + 1) * N]
                ).then_inc(mx_sem, 1)
                vector.wait_ge(mx_sem, b + 1)
                # mask = (t >= 8th largest of its row) -> 1.0 / 0.0
                vector.tensor_scalar(
                    mask[:, b * N : (b + 1) * N],
                    tile[:, b * N : (b + 1) * N],
                    top8[:, 8 * b + 7 : 8 * b + 8],
                    None,
                    mybir.AluOpType.is_ge,
                ).then_inc(v_sem, 1)
    return nc


def kernel(**inputs: np.ndarray) -> np.ndarray:
    global _PROGRAM, LAST_RESULT
    from concourse.bass_utils import run_bass_kernel_spmd

    temp = np.ascontiguousarray(np.asarray(inputs["temp"], dtype=np.float32))
    assert temp.shape == (N, N)

    if _PROGRAM is None:
        _PROGRAM = _build_program()

    in_maps = [
        {"t": np.ascontiguousarray(temp[c * ROWS : (c + 1) * ROWS])}
        for c in range(N_CORES)
    ]
    res = run_bass_kernel_spmd(_PROGRAM, in_maps, list(range(N_CORES)), **RUN_KWARGS)
    LAST_RESULT = res

    mask = np.concatenate([res.results[c]["out"] for c in range(N_CORES)], axis=0)
    return np.ascontiguousarray(np.broadcast_to(mask[None], (B, N, N)))
